# revision 41
# baseline (speedup 1.0000x reference)
"""Bass/Tile TRN2 kernel for a batched self-attention layer.

Reference computation (per batch b, N = 64*64 = 4096 tokens, C = 256, Dp = 32):
    f = input_h @ f_w          [N, Dp]
    g = x @ g_w                [N, Dp]
    s = g @ f.T                [N, N]
    beta = softmax(s, -1)
    o = beta @ input_h         [N, C]
    out = concat([o, x], -1)   [N, 2C]

Sharding: 8 cores = (batch b, query-half) pairs. Each core handles 2048 query
rows of one batch with the full 4096-key attention for that batch.

Design notes (measured ~112.5 us HW exec, rel err 9.6e-4):
  * All layout work (transposes, fp16/bf16 casts, ones-column append) happens
    on the HOST; the device runs only matmuls + exp + normalize.
  * Attention in TRANSPOSED layout per 512-query block, two chunk pairs per
    pipeline step, pipelined ACROSS query blocks: sT[key,q] chunk pairs via
    two concurrent K=32 row-tiled matmuls into double-buffered 2-bank PSUM
    tiles; exp (fp32-range, no max subtraction) straight from PSUM into bf16
    SBUF; PV accumulates exp_chunk.T @ hR_chunk into 4 fp32 PSUM accumulators
    over the 32 key chunks, a ones column yielding the softmax denominator
    for free. Even chunk pairs sit on PE row groups 0/1, odd pairs on 2/3, so
    consecutive QK pairs hit disjoint row groups: their weight loads hide
    under each other's matmuls and a step's 4 QK matmuls run as one ~400ns
    burst (PE tiling-mode switches between QK 32x128 and PV 128x128 drain
    the array, so QK work is batched).
  * The fT/gT projections write into slices of the four o-accumulator PSUM
    banks (a scoped pool's bank handoff would serialize all of proj before
    any attention); each fT span is computed across all four col groups with
    128-col-shifted moving operands so the de-interleave is two
    partition-aligned DVE copies.
  * Input DMAs use large per-partition descriptors (4-8KB) — fine-grained
    span splitting measurably starves the DMA engines. xT and hT go first
    (they gate the projections and therefore attention start); hR is gated
    behind hT via tiny GpSimd touch ops (real WAW dependencies; GpSimd is
    idle and has no FIFO head-of-line blocking). tile_wait_until annotations
    tell the static scheduler the hT halves land late (HBM is shared by all
    8 cores during the ramp) so early attention steps are queued before the
    late fT projections.
  * PE warm-up matmuls + a dummy exp run during the initial DMA so the HAM
    clock gate is at 2.4 GHz and the ACT exp table is loaded when real work
    starts. Normalization (DVE reciprocal + scalar-mul) of each block hides
    in the next block's pipeline.
"""

import numpy as np
import ml_dtypes

import concourse.bass as bass
import concourse.tile as tile
from concourse import bacc
from concourse import mybir
from concourse.bass_utils import run_bass_kernel_spmd

F32 = mybir.dt.float32
F16 = mybir.dt.float16
BF16 = mybir.dt.bfloat16

B, W, C, D = 4, 64, 256, 32
N = W * W                 # 4096 tokens (keys) per batch
NCORES = 8
SHARDS_PER_BATCH = NCORES // B   # 2
NQ = N // SHARDS_PER_BATCH       # 2048 query rows per core
KC = 128                         # key chunk (PE partition dim)
NKC = N // KC                    # 32 key chunks
QBLK = 512                       # query block (moving free dim)
NQB = NQ // QBLK                 # 4 query blocks per core
QSUB = 128                       # query sub-tile (PV stationary M)
NQSUB = QBLK // QSUB             # 4
NP = NKC // 2                    # 16 chunk pairs per query block
NSTEP = NP // 2                  # 8 two-pair pipeline steps
NWARM = 10                       # PE warm-up matmuls during input DMA
Exp = mybir.ActivationFunctionType.Exp


def _build() -> bass.Bass:
    nc = bacc.Bacc("TRN2", target_bir_lowering=False)

    xT = nc.declare_dram_parameter("xT", [C, NQ], F16, isOutput=False)
    hT = nc.declare_dram_parameter("hT", [C, N], F16, isOutput=False)
    hR = nc.declare_dram_parameter("hR", [N, C + 2], BF16, isOutput=False)
    fwg = nc.declare_dram_parameter("fwg", [128, 4 * D], F16, isOutput=False)
    o = nc.declare_dram_parameter("o", [NQ, C], F32, isOutput=True)

    with tile.TileContext(nc) as tc:
        with (
            tc.tile_pool(name="const", bufs=1) as const_pool,
            tc.tile_pool(name="hr", bufs=1) as hr_pool,
            tc.tile_pool(name="inp", bufs=1) as inp_pool,
            tc.tile_pool(name="proj", bufs=1) as proj_pool,
            tc.tile_pool(name="esb", bufs=6) as e_pool,
            tc.tile_pool(name="osb", bufs=4) as out_pool,
            tc.tile_pool(name="rsb", bufs=4) as r_pool,
            tc.tile_pool(name="ops", bufs=1, space="PSUM") as o_pool,
        ):
            zbias = const_pool.tile([128, 1], F32)
            nc.vector.memset(zbias[:, :], 0.0)
            warm = const_pool.tile([128, C + 2], F16)
            nc.vector.memset(warm[:, :], 0.0)
            # Dummy activation pulls the ~2.7us exp table load off the
            # critical path (runs during the input DMA).
            actwarm = const_pool.tile([128, 1], F32)
            nc.scalar.activation(actwarm[:, :], zbias[:, :], Exp, bias=zbias[:, :])

            fwg_sb = const_pool.tile([128, 4 * D], F16)
            nc.sync.dma_start(out=fwg_sb[:, :], in_=fwg[:, :])

            # PE warm-up: junk matmuls on zeroed SBUF while DMA lands; they
            # target the o0 accumulator bank, which attention reuses later.
            wps = o_pool.tile([128, C + 2], F32, tag="o0", name="warm")
            for wi in range(NWARM):
                nc.tensor.matmul(wps[:, :], warm[:, 0:128], warm[:, :], start=True, stop=True)

            xT_sb = [inp_pool.tile([128, NQ], F16, tag=f"xT{cc}", name=f"xT{cc}") for cc in range(2)]
            hT_sb = [inp_pool.tile([128, N], F16, tag=f"hT{cc}", name=f"hT{cc}") for cc in range(2)]
            hr_blk = [
                hr_pool.tile([128, 4, C + 2], BF16, tag=f"hr{p}", name=f"hr{p}")
                for p in range(NKC // 4)
            ]

            # xT and hT as whole-tile transfers: 4-8KB per-partition
            # descriptors measurably beat smaller pieces on aggregate DMA
            # throughput (each dma_start's descriptors already spread across
            # the 16 queues). hR is gated behind them (below).
            # Only qb0's xT piece gates attention start; the rest rides with hR.
            for cc in range(2):
                nc.sync.dma_start(
                    out=xT_sb[cc][:, 0:QBLK], in_=xT[cc * 128:(cc + 1) * 128, 0:QBLK]
                )
            # tile_wait_until = modeled-time annotation only (no runtime
            # wait): tells the static scheduler the hT halves land late (HBM
            # is shared by all 8 cores during the ramp), so early attention
            # steps get queued BEFORE the late fT projections instead of
            # stalling behind them.
            for h in range(2):
                with tc.tile_wait_until(0.006 + 0.005 * h):
                    for cc in range(2):
                        nc.sync.dma_start(
                            out=hT_sb[cc][:, h * 2048:(h + 1) * 2048],
                            in_=hT[cc * 128:(cc + 1) * 128, h * 2048:(h + 1) * 2048],
                        )

            def dma_gated_inputs():
                # GpSimd touches (idle engine, so no FIFO head-of-line
                # blocking) sequence the hR transfers after the hT spans.
                gate = hT_sb[0][0:1, N - 1:N]
                for cc in range(2):
                    nc.gpsimd.tensor_copy(xT_sb[cc][0:1, QBLK:QBLK + 1], gate)
                    nc.sync.dma_start(
                        out=xT_sb[cc][:, QBLK:], in_=xT[cc * 128:(cc + 1) * 128, QBLK:]
                    )
                for p in range(NKC // 4):
                    nc.gpsimd.tensor_copy(hr_blk[p][0:1, 0:1, 0:1], gate)
                    # Host pre-permuted: chunk k = 4*blk + j holds keys 128k..128k+127.
                    nc.sync.dma_start(
                        out=hr_blk[p][:, :, :],
                        in_=hR[p * 512:(p + 1) * 512, :].rearrange("(p j) c -> p j c", p=128),
                    )

            # fT/gT in fp16. Chunk pair g uses PE row groups 0/1 when g is
            # even, 2/3 when odd (chunk 2g+i on rows 64*(g%2)+32*i):
            # consecutive QK pairs then touch disjoint row groups, so their
            # weight loads overlap each other's matmuls. gT is duplicated on
            # all four row groups. fT4 col s holds pairs 2s (rows 0:64) and
            # 2s+1 (rows 64:128).
            fT4_sb = proj_pool.tile([128, 8, 128], F16)
            gT4_sb = proj_pool.tile([128, NQB, QBLK], F16)

            # The projections borrow the four o-accumulator PSUM banks (in
            # 256-column half-span pieces that fit the 258-column tiles)
            # instead of a scoped pool: a scoped pool's bank handoff to the
            # attention s-pool would serialize ALL of proj before ANY
            # attention work. Rotating o-tags keeps four pieces in flight.
            _ocnt = [0]

            def otile(name):
                _ocnt[0] += 1
                return o_pool.tile([128, C + 2], F32, tag=f"o{_ocnt[0] % 4}", name=name)

            def proj_g(qb):
                for hq in range(2):
                    st = otile(f"gp{qb}_{hq}")
                    q0 = qb * QBLK + hq * 256
                    for i in range(4):
                        for cc in range(2):
                            nc.tensor.matmul(
                                st[32 * i:32 * (i + 1), 0:256],
                                fwg_sb[:, cc * 2 * D + D:cc * 2 * D + 2 * D],
                                xT_sb[cc][:, q0:q0 + 256],
                                start=(cc == 0),
                                stop=(cc == 1),
                                tile_position=(0, 32 * i),
                            )
                    nc.vector.tensor_copy(
                        gT4_sb[:, qb, hq * 256:(hq + 1) * 256], st[:, 0:256]
                    )

            def proj_f(s):
                # One tile computes all four chunks of span s across the four
                # col groups; group i reads a 128i-shifted moving operand so
                # chunk 4s+i lands in cols 0:128 of rows 32i — the
                # de-interleave is two partition-aligned [64,128] copies.
                # (The very last group would read past the end of hT, so it
                # uses an N=128 matmul instead.)
                st = otile(f"fp{s}")
                for i in range(4):
                    m0 = s * 512 + 128 * i
                    nw = 256 if m0 + 256 <= N else 128
                    for cc in range(2):
                        nc.tensor.matmul(
                            st[32 * i:32 * (i + 1), 0:nw],
                            fwg_sb[:, cc * 2 * D:cc * 2 * D + D],
                            hT_sb[cc][:, m0:m0 + nw],
                            start=(cc == 0),
                            stop=(cc == 1),
                            tile_position=(0, 32 * i),
                        )
                nc.vector.tensor_copy(fT4_sb[0:64, s, :], st[0:64, 0:128])
                nc.vector.tensor_copy(fT4_sb[64:128, s, :], st[64:128, 0:128])

            proj_g(0)
            for s in range(8):
                proj_f(s)
            dma_gated_inputs()
            for qb in range(1, NQB):
                proj_g(qb)

            def pv(o_ps, e_ap, k):
                for i in range(NQSUB):
                    nc.tensor.matmul(
                        o_ps[i][:, :],
                        e_ap[:, i * 128:(i + 1) * 128],
                        hr_blk[k // 4][:, k % 4, :],
                        start=(k == 0),
                        stop=(k == NKC - 1),
                    )

            def norm_out(qb, o_ps):
                for i in range(NQSUB):
                    rec = r_pool.tile([128, 1], F32, tag="rec", name=f"rec{qb}_{i}")
                    nc.vector.reciprocal(rec[:, :], o_ps[i][:, C:C + 1])
                    out_sb = out_pool.tile([128, C], F32, tag="ob", name=f"ob{qb}_{i}")
                    nc.vector.tensor_scalar_mul(out_sb[:, :], o_ps[i][:, 0:C], rec[:, :])
                    r0 = qb * QBLK + i * 128
                    nc.sync.dma_start(out=o[r0:r0 + 128, :], in_=out_sb[:, :])

            # --- attention: steps of two chunk pairs, pipelined ACROSS query
            # blocks (the QK prefetch crosses qblock boundaries, so the PE
            # never drains between blocks).
            # step pipeline: [QK pair, QK pair](t+1) -> [exp, exp](t) -> [16x PV](t)
            with tc.tile_pool(name="sps", bufs=2, space="PSUM") as s_pool:
                def qk_pair(p):
                    qb, g = divmod(p, NP)
                    s_ps = s_pool.tile([128, 2, QBLK], F32, tag="s", name=f"sps{qb}_{g}")
                    r0 = 64 * (g % 2)
                    for half in range(2):
                        rb = r0 + 32 * half
                        nc.tensor.matmul(
                            s_ps[:, half, :],
                            fT4_sb[rb:rb + 32, g // 2, :],
                            gT4_sb[rb:rb + 32, qb, :],
                            start=True,
                            stop=True,
                            tile_position=(rb, 0),
                        )
                    return s_ps

                NPAIRS = NQB * NP
                o_ps = None
                prev = [(0, qk_pair(0)), (1, qk_pair(1))]
                for t in range(NPAIRS // 2):
                    nxt = None
                    if 2 * t + 2 < NPAIRS:
                        nxt = [(2 * t + 2, qk_pair(2 * t + 2)), (2 * t + 3, qk_pair(2 * t + 3))]
                    es = []
                    for p, s_ps in prev:
                        qb, g = divmod(p, NP)
                        e_sb = e_pool.tile([128, 2, QBLK], BF16, tag="e", name=f"e{qb}_{g}")
                        nc.scalar.activation(e_sb[:, :, :], s_ps[:, :, :], Exp, bias=zbias[:, :])
                        es.append((p, e_sb))
                    for p, e in es:
                        qb, g = divmod(p, NP)
                        if g == 0:
                            o_ps = [
                                o_pool.tile([128, C + 2], F32, tag=f"o{i}", name=f"ops{qb}_{i}")
                                for i in range(NQSUB)
                            ]
                        for half in range(2):
                            pv(o_ps, e[:, half, :], 2 * g + half)
                        if g == NP - 1:
                            norm_out(qb, o_ps)
                    prev = nxt

    nc.finalize()
    return nc


_CACHE: dict = {}


def _get_nc() -> bass.Bass:
    if "nc" not in _CACHE:
        _CACHE["nc"] = _build()
    return _CACHE["nc"]


def _prep_batch(hf_b):
    """Per-batch host prep shared by both query-half cores."""
    hT = np.ascontiguousarray(hf_b.T.astype(np.float16))              # [C, N]
    aug = np.empty((N, C + 2), dtype=ml_dtypes.bfloat16)
    aug[:, 0:C] = hf_b.astype(ml_dtypes.bfloat16)
    aug[:, C] = 1.0
    aug[:, C + 1] = 0.0
    # chunk k = 4*blk + j holds keys 128k..128k+127: [blk, j, p, c] -> [blk, p, j, c]
    hR = np.ascontiguousarray(
        aug.reshape(NKC // 4, 4, 128, C + 2).transpose(0, 2, 1, 3).reshape(N, C + 2)
    )
    return hT, hR


def _shard(x, input_h, f_w, g_w):
    xf = np.asarray(x, dtype=np.float32).reshape(B, N, C)
    hf = np.asarray(input_h, dtype=np.float32).reshape(B, N, C)
    fwf = np.asarray(f_w, dtype=np.float32).reshape(C, D)
    gwf = np.asarray(g_w, dtype=np.float32).reshape(C, D)
    fwg = np.empty((128, 4 * D), dtype=np.float16)
    for cc in range(2):
        fwg[:, cc * 2 * D:cc * 2 * D + D] = fwf[cc * 128:(cc + 1) * 128, :]
        fwg[:, cc * 2 * D + D:cc * 2 * D + 2 * D] = gwf[cc * 128:(cc + 1) * 128, :]
    per_batch = [_prep_batch(hf[b]) for b in range(B)]
    in_maps = []
    for c in range(NCORES):
        b, half = divmod(c, SHARDS_PER_BATCH)
        hTb, hRb = per_batch[b]
        xTc = np.ascontiguousarray(
            xf[b, half * NQ:(half + 1) * NQ].T.astype(np.float16)
        )
        in_maps.append({"xT": xTc, "hT": hTb, "hR": hRb, "fwg": fwg})
    return in_maps


def _gather(results, x):
    of = np.empty((B, N, C), np.float32)
    for c in range(NCORES):
        b, half = divmod(c, SHARDS_PER_BATCH)
        of[b, half * NQ:(half + 1) * NQ] = results[c]["o"]
    o4 = of.reshape(B, W, W, C)
    x4 = np.asarray(x, dtype=np.float32).reshape(B, W, W, C)
    return np.concatenate([o4, x4], axis=-1)


def run(inputs: dict, trace: bool = False):
    """Run the kernel; returns (full_output, BassKernelResults)."""
    in_maps = _shard(**inputs)
    res = run_bass_kernel_spmd(_get_nc(), in_maps, list(range(NCORES)), trace=trace)
    out = _gather(res.results, inputs["x"])
    return out, res


def kernel(**inputs) -> np.ndarray:
    out, _ = run(inputs, trace=False)
    return out


# revision 45
# speedup vs baseline: 1.1659x; 1.1659x over previous
"""Bass/Tile TRN2 kernel for a batched self-attention layer.

Reference computation (per batch b, N = 64*64 = 4096 tokens, C = 256, Dp = 32):
    f = input_h @ f_w          [N, Dp]
    g = x @ g_w                [N, Dp]
    s = g @ f.T                [N, N]
    beta = softmax(s, -1)
    o = beta @ input_h         [N, C]
    out = concat([o, x], -1)   [N, 2C]

Sharding: 8 cores = (batch b, query-half) pairs. Each core handles 2048 query
rows of one batch with the full 4096-key attention for that batch.

Design notes (measured ~112.5 us HW exec, rel err 9.6e-4):
  * All layout work (transposes, fp16/bf16 casts, ones-column append) happens
    on the HOST; the device runs only matmuls + exp + normalize.
  * Attention in TRANSPOSED layout per 512-query block, two chunk pairs per
    pipeline step, pipelined ACROSS query blocks: sT[key,q] chunk pairs via
    two concurrent K=32 row-tiled matmuls into double-buffered 2-bank PSUM
    tiles; exp (fp32-range, no max subtraction) straight from PSUM into bf16
    SBUF; PV accumulates exp_chunk.T @ hR_chunk into 4 fp32 PSUM accumulators
    over the 32 key chunks, a ones column yielding the softmax denominator
    for free. Even chunk pairs sit on PE row groups 0/1, odd pairs on 2/3, so
    consecutive QK pairs hit disjoint row groups: their weight loads hide
    under each other's matmuls and a step's 4 QK matmuls run as one ~400ns
    burst (PE tiling-mode switches between QK 32x128 and PV 128x128 drain
    the array, so QK work is batched).
  * The fT/gT projections write into slices of the four o-accumulator PSUM
    banks (a scoped pool's bank handoff would serialize all of proj before
    any attention); each fT span is computed across all four col groups with
    128-col-shifted moving operands so the de-interleave is two
    partition-aligned DVE copies.
  * Input DMAs use large per-partition descriptors (4-8KB) — fine-grained
    span splitting measurably starves the DMA engines. xT and hT go first
    (they gate the projections and therefore attention start); hR is gated
    behind hT via tiny GpSimd touch ops (real WAW dependencies; GpSimd is
    idle and has no FIFO head-of-line blocking). tile_wait_until annotations
    tell the static scheduler the hT halves land late (HBM is shared by all
    8 cores during the ramp) so early attention steps are queued before the
    late fT projections.
  * PE warm-up matmuls + a dummy exp run during the initial DMA so the HAM
    clock gate is at 2.4 GHz and the ACT exp table is loaded when real work
    starts. Normalization (DVE reciprocal + scalar-mul) of each block hides
    in the next block's pipeline.
"""

import numpy as np
import ml_dtypes

import concourse.bass as bass
import concourse.tile as tile
from concourse import bacc
from concourse import mybir
from concourse.bass_utils import run_bass_kernel_spmd

F32 = mybir.dt.float32
F16 = mybir.dt.float16
BF16 = mybir.dt.bfloat16

B, W, C, D = 4, 64, 256, 32
N = W * W                 # 4096 tokens (keys) per batch
NCORES = 8
SHARDS_PER_BATCH = NCORES // B   # 2
NQ = N // SHARDS_PER_BATCH       # 2048 query rows per core
KC = 128                         # key chunk (PE partition dim)
NKC = N // KC                    # 32 key chunks
QBLK = 512                       # query block (moving free dim)
NQB = NQ // QBLK                 # 4 query blocks per core
QSUB = 128                       # query sub-tile (PV stationary M)
NQSUB = QBLK // QSUB             # 4
NP = NKC // 2                    # 16 chunk pairs per query block
NSTEP = NP // 2                  # 8 two-pair pipeline steps
NWARM = 10                       # PE warm-up matmuls during input DMA
Exp = mybir.ActivationFunctionType.Exp


def _build() -> bass.Bass:
    nc = bacc.Bacc("TRN2", target_bir_lowering=False)

    xT = nc.declare_dram_parameter("xT", [C, NQ], F16, isOutput=False)
    hT = nc.declare_dram_parameter("hT", [C, N], F16, isOutput=False)
    hR = nc.declare_dram_parameter("hR", [N, C + 2], BF16, isOutput=False)
    fwg = nc.declare_dram_parameter("fwg", [128, 4 * D], F16, isOutput=False)
    o = nc.declare_dram_parameter("o", [NQ, C], F32, isOutput=True)

    with tile.TileContext(nc) as tc:
        with (
            tc.tile_pool(name="const", bufs=1) as const_pool,
            tc.tile_pool(name="hr", bufs=1) as hr_pool,
            tc.tile_pool(name="inp", bufs=1) as inp_pool,
            tc.tile_pool(name="proj", bufs=1) as proj_pool,
            tc.tile_pool(name="esb", bufs=4) as e_pool,
            tc.tile_pool(name="osb", bufs=4) as out_pool,
            tc.tile_pool(name="rsb", bufs=4) as r_pool,
            tc.tile_pool(name="ops", bufs=1, space="PSUM") as o_pool,
        ):
            zbias = const_pool.tile([128, 1], F32)
            nc.vector.memset(zbias[:, :], 0.0)
            warm = const_pool.tile([128, C + 2], F16)
            nc.vector.memset(warm[:, :], 0.0)
            # Dummy activation pulls the ~2.7us exp table load off the
            # critical path (runs during the input DMA).
            actwarm = const_pool.tile([128, 1], F32)
            nc.scalar.activation(actwarm[:, :], zbias[:, :], Exp, bias=zbias[:, :])

            fwg_sb = const_pool.tile([128, 4 * D], F16)
            nc.sync.dma_start(out=fwg_sb[:, :], in_=fwg[:, :])

            # PE warm-up: junk matmuls on zeroed SBUF while DMA lands; they
            # target the o0 accumulator bank, which attention reuses later.
            wps = o_pool.tile([128, C + 2], F32, tag="o0", name="warm")
            for wi in range(NWARM):
                nc.tensor.matmul(wps[:, :], warm[:, 0:128], warm[:, :], start=True, stop=True)

            xT_sb = [inp_pool.tile([128, NQ], F16, tag=f"xT{cc}", name=f"xT{cc}") for cc in range(2)]
            hT_sb = [inp_pool.tile([128, N], F16, tag=f"hT{cc}", name=f"hT{cc}") for cc in range(2)]
            hr_blk = [
                hr_pool.tile([128, 4, C + 2], BF16, tag=f"hr{p}", name=f"hr{p}")
                for p in range(NKC // 4)
            ]

            # xT and hT as whole-tile transfers: 4-8KB per-partition
            # descriptors measurably beat smaller pieces on aggregate DMA
            # throughput (each dma_start's descriptors already spread across
            # the 16 queues). hR is gated behind them (below).
            for cc in range(2):
                nc.sync.dma_start(out=xT_sb[cc][:, :], in_=xT[cc * 128:(cc + 1) * 128, :])
            # tile_wait_until = modeled-time annotation only (no runtime
            # wait): tells the static scheduler the hT halves land late (HBM
            # is shared by all 8 cores during the ramp), so early attention
            # steps get queued BEFORE the late fT projections instead of
            # stalling behind them.
            for h in range(2):
                with tc.tile_wait_until(0.006 + 0.005 * h):
                    for cc in range(2):
                        nc.sync.dma_start(
                            out=hT_sb[cc][:, h * 2048:(h + 1) * 2048],
                            in_=hT[cc * 128:(cc + 1) * 128, h * 2048:(h + 1) * 2048],
                        )

            def dma_gated_inputs():
                # GpSimd touches (idle engine, so no FIFO head-of-line
                # blocking) sequence the hR transfers after the hT spans.
                gate = hT_sb[0][0:1, N - 1:N]
                for p in range(NKC // 4):
                    nc.gpsimd.tensor_copy(hr_blk[p][0:1, 0:1, 0:1], gate)
                    # Host pre-permuted: chunk k = 4*blk + j holds keys 128k..128k+127.
                    nc.sync.dma_start(
                        out=hr_blk[p][:, :, :],
                        in_=hR[p * 512:(p + 1) * 512, :].rearrange("(p j) c -> p j c", p=128),
                    )

            # fT/gT in fp16. Chunk pair g uses PE row groups 0/1 when g is
            # even, 2/3 when odd (chunk 2g+i on rows 64*(g%2)+32*i):
            # consecutive QK pairs then touch disjoint row groups, so their
            # weight loads overlap each other's matmuls. gT is duplicated on
            # all four row groups. fT4 col s holds pairs 2s (rows 0:64) and
            # 2s+1 (rows 64:128).
            fT4_sb = proj_pool.tile([128, 8, 128], F16)
            gT4_sb = proj_pool.tile([128, NQB, QBLK], F16)

            # The projections borrow the four o-accumulator PSUM banks (in
            # 256-column half-span pieces that fit the 258-column tiles)
            # instead of a scoped pool: a scoped pool's bank handoff to the
            # attention s-pool would serialize ALL of proj before ANY
            # attention work. Rotating o-tags keeps four pieces in flight.
            _ocnt = [0]

            def otile(name):
                _ocnt[0] += 1
                return o_pool.tile([128, C + 2], F32, tag=f"o{_ocnt[0] % 4}", name=name)

            def proj_g(qb):
                for hq in range(2):
                    st = otile(f"gp{qb}_{hq}")
                    q0 = qb * QBLK + hq * 256
                    for i in range(4):
                        for cc in range(2):
                            nc.tensor.matmul(
                                st[32 * i:32 * (i + 1), 0:256],
                                fwg_sb[:, cc * 2 * D + D:cc * 2 * D + 2 * D],
                                xT_sb[cc][:, q0:q0 + 256],
                                start=(cc == 0),
                                stop=(cc == 1),
                                tile_position=(0, 32 * i),
                            )
                    nc.vector.tensor_copy(
                        gT4_sb[:, qb, hq * 256:(hq + 1) * 256], st[:, 0:256]
                    )

            def proj_f(s):
                # One tile computes all four chunks of span s across the four
                # col groups; group i reads a 128i-shifted moving operand so
                # chunk 4s+i lands in cols 0:128 of rows 32i — the
                # de-interleave is two partition-aligned [64,128] copies.
                # (The very last group would read past the end of hT, so it
                # uses an N=128 matmul instead.)
                st = otile(f"fp{s}")
                for i in range(4):
                    m0 = s * 512 + 128 * i
                    nw = 256 if m0 + 256 <= N else 128
                    for cc in range(2):
                        nc.tensor.matmul(
                            st[32 * i:32 * (i + 1), 0:nw],
                            fwg_sb[:, cc * 2 * D:cc * 2 * D + D],
                            hT_sb[cc][:, m0:m0 + nw],
                            start=(cc == 0),
                            stop=(cc == 1),
                            tile_position=(0, 32 * i),
                        )
                nc.vector.tensor_copy(fT4_sb[0:64, s, :], st[0:64, 0:128])
                nc.vector.tensor_copy(fT4_sb[64:128, s, :], st[64:128, 0:128])

            proj_g(0)
            for s in range(8):
                proj_f(s)
            for qb in range(1, NQB):
                proj_g(qb)
            dma_gated_inputs()

            def pv(o_ps, e_ap, k):
                for i in range(NQSUB):
                    nc.tensor.matmul(
                        o_ps[i][:, :],
                        e_ap[:, i * 128:(i + 1) * 128],
                        hr_blk[k // 4][:, k % 4, :],
                        start=(k == 0),
                        stop=(k == NKC - 1),
                    )

            def norm_out(qb, o_ps):
                for i in range(NQSUB):
                    rec = r_pool.tile([128, 1], F32, tag="rec", name=f"rec{qb}_{i}")
                    nc.vector.reciprocal(rec[:, :], o_ps[i][:, C:C + 1])
                    out_sb = out_pool.tile([128, C], F32, tag="ob", name=f"ob{qb}_{i}")
                    nc.vector.tensor_scalar_mul(out_sb[:, :], o_ps[i][:, 0:C], rec[:, :])
                    r0 = qb * QBLK + i * 128
                    nc.sync.dma_start(out=o[r0:r0 + 128, :], in_=out_sb[:, :])

            # --- attention: steps of two chunk pairs, pipelined ACROSS query
            # blocks (the QK prefetch crosses qblock boundaries, so the PE
            # never drains between blocks).
            # step pipeline: [QK pair, QK pair](t+1) -> [exp, exp](t) -> [16x PV](t)
            with tc.tile_pool(name="sps", bufs=2, space="PSUM") as s_pool:
                def qk_pair(p):
                    qb, g = divmod(p, NP)
                    s_ps = s_pool.tile([128, 2, QBLK], F32, tag="s", name=f"sps{qb}_{g}")
                    r0 = 64 * (g % 2)
                    for half in range(2):
                        rb = r0 + 32 * half
                        nc.tensor.matmul(
                            s_ps[:, half, :],
                            fT4_sb[rb:rb + 32, g // 2, :],
                            gT4_sb[rb:rb + 32, qb, :],
                            start=True,
                            stop=True,
                            tile_position=(rb, 0),
                        )
                    return s_ps

                NPAIRS = NQB * NP
                o_ps = None
                prev = [(0, qk_pair(0)), (1, qk_pair(1))]
                for t in range(NPAIRS // 2):
                    nxt = None
                    if 2 * t + 2 < NPAIRS:
                        nxt = [(2 * t + 2, qk_pair(2 * t + 2)), (2 * t + 3, qk_pair(2 * t + 3))]
                    es = []
                    for p, s_ps in prev:
                        qb, g = divmod(p, NP)
                        e_sb = e_pool.tile([128, 2, QBLK], BF16, tag="e", name=f"e{qb}_{g}")
                        nc.scalar.activation(e_sb[:, :, :], s_ps[:, :, :], Exp, bias=zbias[:, :])
                        es.append((p, e_sb))
                    for p, e in es:
                        qb, g = divmod(p, NP)
                        if g == 0:
                            o_ps = [
                                o_pool.tile([128, C + 2], F32, tag=f"o{i}", name=f"ops{qb}_{i}")
                                for i in range(NQSUB)
                            ]
                        for half in range(2):
                            pv(o_ps, e[:, half, :], 2 * g + half)
                        if g == NP - 1:
                            norm_out(qb, o_ps)
                    prev = nxt

    nc.finalize()
    return nc


_CACHE: dict = {}


def _get_nc() -> bass.Bass:
    if "nc" not in _CACHE:
        _CACHE["nc"] = _build()
    return _CACHE["nc"]


def _prep_batch(hf_b):
    """Per-batch host prep shared by both query-half cores."""
    hT = np.ascontiguousarray(hf_b.T.astype(np.float16))              # [C, N]
    aug = np.empty((N, C + 2), dtype=ml_dtypes.bfloat16)
    aug[:, 0:C] = hf_b.astype(ml_dtypes.bfloat16)
    aug[:, C] = 1.0
    aug[:, C + 1] = 0.0
    # chunk k = 4*blk + j holds keys 128k..128k+127: [blk, j, p, c] -> [blk, p, j, c]
    hR = np.ascontiguousarray(
        aug.reshape(NKC // 4, 4, 128, C + 2).transpose(0, 2, 1, 3).reshape(N, C + 2)
    )
    return hT, hR


def _shard(x, input_h, f_w, g_w):
    xf = np.asarray(x, dtype=np.float32).reshape(B, N, C)
    hf = np.asarray(input_h, dtype=np.float32).reshape(B, N, C)
    fwf = np.asarray(f_w, dtype=np.float32).reshape(C, D)
    gwf = np.asarray(g_w, dtype=np.float32).reshape(C, D)
    fwg = np.empty((128, 4 * D), dtype=np.float16)
    for cc in range(2):
        fwg[:, cc * 2 * D:cc * 2 * D + D] = fwf[cc * 128:(cc + 1) * 128, :]
        fwg[:, cc * 2 * D + D:cc * 2 * D + 2 * D] = gwf[cc * 128:(cc + 1) * 128, :]
    per_batch = [_prep_batch(hf[b]) for b in range(B)]
    in_maps = []
    for c in range(NCORES):
        b, half = divmod(c, SHARDS_PER_BATCH)
        hTb, hRb = per_batch[b]
        xTc = np.ascontiguousarray(
            xf[b, half * NQ:(half + 1) * NQ].T.astype(np.float16)
        )
        in_maps.append({"xT": xTc, "hT": hTb, "hR": hRb, "fwg": fwg})
    return in_maps


def _gather(results, x):
    of = np.empty((B, N, C), np.float32)
    for c in range(NCORES):
        b, half = divmod(c, SHARDS_PER_BATCH)
        of[b, half * NQ:(half + 1) * NQ] = results[c]["o"]
    o4 = of.reshape(B, W, W, C)
    x4 = np.asarray(x, dtype=np.float32).reshape(B, W, W, C)
    return np.concatenate([o4, x4], axis=-1)


def run(inputs: dict, trace: bool = False):
    """Run the kernel; returns (full_output, BassKernelResults)."""
    in_maps = _shard(**inputs)
    res = run_bass_kernel_spmd(_get_nc(), in_maps, list(range(NCORES)), trace=trace)
    out = _gather(res.results, inputs["x"])
    return out, res


def kernel(**inputs) -> np.ndarray:
    out, _ = run(inputs, trace=False)
    return out


# revision 50
# speedup vs baseline: 1.1842x; 1.0157x over previous
"""Bass/Tile TRN2 kernel for a batched self-attention layer.

Reference computation (per batch b, N = 64*64 = 4096 tokens, C = 256, Dp = 32):
    f = input_h @ f_w          [N, Dp]
    g = x @ g_w                [N, Dp]
    s = g @ f.T                [N, N]
    beta = softmax(s, -1)
    o = beta @ input_h         [N, C]
    out = concat([o, x], -1)   [N, 2C]

Sharding: 8 cores = (batch b, query-half) pairs. Each core handles 2048 query
rows of one batch with the full 4096-key attention for that batch.

Design notes (measured ~112.5 us HW exec, rel err 9.6e-4):
  * All layout work (transposes, fp16/bf16 casts, ones-column append) happens
    on the HOST; the device runs only matmuls + exp + normalize.
  * Attention in TRANSPOSED layout per 512-query block, two chunk pairs per
    pipeline step, pipelined ACROSS query blocks: sT[key,q] chunk pairs via
    two concurrent K=32 row-tiled matmuls into double-buffered 2-bank PSUM
    tiles; exp (fp32-range, no max subtraction) straight from PSUM into bf16
    SBUF; PV accumulates exp_chunk.T @ hR_chunk into 4 fp32 PSUM accumulators
    over the 32 key chunks, a ones column yielding the softmax denominator
    for free. Even chunk pairs sit on PE row groups 0/1, odd pairs on 2/3, so
    consecutive QK pairs hit disjoint row groups: their weight loads hide
    under each other's matmuls and a step's 4 QK matmuls run as one ~400ns
    burst (PE tiling-mode switches between QK 32x128 and PV 128x128 drain
    the array, so QK work is batched).
  * The fT/gT projections write into slices of the four o-accumulator PSUM
    banks (a scoped pool's bank handoff would serialize all of proj before
    any attention); each fT span is computed across all four col groups with
    128-col-shifted moving operands so the de-interleave is two
    partition-aligned DVE copies.
  * Input DMAs use large per-partition descriptors (4-8KB) — fine-grained
    span splitting measurably starves the DMA engines. xT and hT go first
    (they gate the projections and therefore attention start); hR is gated
    behind hT via tiny GpSimd touch ops (real WAW dependencies; GpSimd is
    idle and has no FIFO head-of-line blocking). tile_wait_until annotations
    tell the static scheduler the hT halves land late (HBM is shared by all
    8 cores during the ramp) so early attention steps are queued before the
    late fT projections.
  * PE warm-up matmuls + a dummy exp run during the initial DMA so the HAM
    clock gate is at 2.4 GHz and the ACT exp table is loaded when real work
    starts. Normalization (DVE reciprocal + scalar-mul) of each block hides
    in the next block's pipeline.
"""

import numpy as np
import ml_dtypes

import concourse.bass as bass
import concourse.tile as tile
from concourse import bacc
from concourse import mybir
from concourse.bass_utils import run_bass_kernel_spmd

F32 = mybir.dt.float32
F16 = mybir.dt.float16
BF16 = mybir.dt.bfloat16

B, W, C, D = 4, 64, 256, 32
N = W * W                 # 4096 tokens (keys) per batch
NCORES = 8
SHARDS_PER_BATCH = NCORES // B   # 2
NQ = N // SHARDS_PER_BATCH       # 2048 query rows per core
KC = 128                         # key chunk (PE partition dim)
NKC = N // KC                    # 32 key chunks
QBLK = 512                       # query block (moving free dim)
NQB = NQ // QBLK                 # 4 query blocks per core
QSUB = 128                       # query sub-tile (PV stationary M)
NQSUB = QBLK // QSUB             # 4
NP = NKC // 2                    # 16 chunk pairs per query block
NSTEP = NP // 2                  # 8 two-pair pipeline steps
NWARM = 10                       # PE warm-up matmuls during input DMA
Exp = mybir.ActivationFunctionType.Exp


def _build() -> bass.Bass:
    nc = bacc.Bacc("TRN2", target_bir_lowering=False)

    xT = nc.declare_dram_parameter("xT", [C, NQ], F16, isOutput=False)
    hT = nc.declare_dram_parameter("hT", [C, N], F16, isOutput=False)
    hR = nc.declare_dram_parameter("hR", [N, C + 2], BF16, isOutput=False)
    fwg = nc.declare_dram_parameter("fwg", [128, 4 * D], F16, isOutput=False)
    o = nc.declare_dram_parameter("o", [NQ, C], F32, isOutput=True)

    with tile.TileContext(nc) as tc:
        with (
            tc.tile_pool(name="const", bufs=1) as const_pool,
            tc.tile_pool(name="hr", bufs=1) as hr_pool,
            tc.tile_pool(name="inp", bufs=1) as inp_pool,
            tc.tile_pool(name="proj", bufs=1) as proj_pool,
            tc.tile_pool(name="esb", bufs=4) as e_pool,
            tc.tile_pool(name="osb", bufs=4) as out_pool,
            tc.tile_pool(name="rsb", bufs=4) as r_pool,
            tc.tile_pool(name="ops", bufs=1, space="PSUM") as o_pool,
        ):
            zbias = const_pool.tile([128, 1], F32)
            nc.vector.memset(zbias[:, :], 0.0)
            warm = const_pool.tile([128, C + 2], F16)
            nc.vector.memset(warm[:, :], 0.0)
            # Dummy activation pulls the ~2.7us exp table load off the
            # critical path (runs during the input DMA).
            actwarm = const_pool.tile([128, 1], F32)
            nc.scalar.activation(actwarm[:, :], zbias[:, :], Exp, bias=zbias[:, :])

            fwg_sb = const_pool.tile([128, 4 * D], F16)
            nc.sync.dma_start(out=fwg_sb[:, :], in_=fwg[:, :])

            # PE warm-up: junk matmuls on zeroed SBUF while DMA lands; they
            # target the o0 accumulator bank, which attention reuses later.
            wps = o_pool.tile([128, C + 2], F32, tag="o0", name="warm")
            for wi in range(NWARM):
                nc.tensor.matmul(wps[:, :], warm[:, 0:128], warm[:, :], start=True, stop=True)

            xT_sb = [inp_pool.tile([128, NQ], F16, tag=f"xT{cc}", name=f"xT{cc}") for cc in range(2)]
            hT_sb = [inp_pool.tile([128, N], F16, tag=f"hT{cc}", name=f"hT{cc}") for cc in range(2)]
            hr_blk = [
                hr_pool.tile([128, 4, C + 2], BF16, tag=f"hr{p}", name=f"hr{p}")
                for p in range(NKC // 4)
            ]

            # xT and hT as whole-tile transfers: 4-8KB per-partition
            # descriptors measurably beat smaller pieces on aggregate DMA
            # throughput (each dma_start's descriptors already spread across
            # the 16 queues). hR is gated behind them (below).
            # Only qb0's xT piece gates attention start; the rest is gated
            # behind hT (below) and consumed mid-attention by proj_g2.
            for cc in range(2):
                nc.sync.dma_start(
                    out=xT_sb[cc][:, 0:QBLK], in_=xT[cc * 128:(cc + 1) * 128, 0:QBLK]
                )
            # tile_wait_until = modeled-time annotation only (no runtime
            # wait): tells the static scheduler the hT halves land late (HBM
            # is shared by all 8 cores during the ramp), so early attention
            # steps get queued BEFORE the late fT projections instead of
            # stalling behind them.
            for h in range(2):
                with tc.tile_wait_until(0.006 + 0.005 * h):
                    for cc in range(2):
                        nc.sync.dma_start(
                            out=hT_sb[cc][:, h * 2048:(h + 1) * 2048],
                            in_=hT[cc * 128:(cc + 1) * 128, h * 2048:(h + 1) * 2048],
                        )

            def dma_gated_inputs():
                # GpSimd touches (idle engine, so no FIFO head-of-line
                # blocking) sequence the hR transfers after the hT spans.
                gate = hT_sb[0][0:1, N - 1:N]
                for cc in range(2):
                    nc.gpsimd.tensor_copy(xT_sb[cc][0:1, QBLK:QBLK + 1], gate)
                    nc.sync.dma_start(
                        out=xT_sb[cc][:, QBLK:], in_=xT[cc * 128:(cc + 1) * 128, QBLK:]
                    )
                for p in range(NKC // 4):
                    nc.gpsimd.tensor_copy(hr_blk[p][0:1, 0:1, 0:1], gate)
                    # Host pre-permuted: chunk k = 4*blk + j holds keys 128k..128k+127.
                    nc.sync.dma_start(
                        out=hr_blk[p][:, :, :],
                        in_=hR[p * 512:(p + 1) * 512, :].rearrange("(p j) c -> p j c", p=128),
                    )

            # fT/gT in fp16. Chunk pair g uses PE row groups 0/1 when g is
            # even, 2/3 when odd (chunk 2g+i on rows 64*(g%2)+32*i):
            # consecutive QK pairs then touch disjoint row groups, so their
            # weight loads overlap each other's matmuls. gT is duplicated on
            # all four row groups. fT4 col s holds pairs 2s (rows 0:64) and
            # 2s+1 (rows 64:128).
            fT4_sb = proj_pool.tile([128, 8, 128], F16)
            gT4_sb = proj_pool.tile([128, NQB, QBLK], F16)

            # The projections borrow the four o-accumulator PSUM banks (in
            # 256-column half-span pieces that fit the 258-column tiles)
            # instead of a scoped pool: a scoped pool's bank handoff to the
            # attention s-pool would serialize ALL of proj before ANY
            # attention work. Rotating o-tags keeps four pieces in flight.
            _ocnt = [0]

            def otile(name):
                _ocnt[0] += 1
                return o_pool.tile([128, C + 2], F32, tag=f"o{_ocnt[0] % 4}", name=name)

            def proj_g(qb):
                for hq in range(2):
                    st = otile(f"gp{qb}_{hq}")
                    q0 = qb * QBLK + hq * 256
                    for i in range(4):
                        for cc in range(2):
                            nc.tensor.matmul(
                                st[32 * i:32 * (i + 1), 0:256],
                                fwg_sb[:, cc * 2 * D + D:cc * 2 * D + 2 * D],
                                xT_sb[cc][:, q0:q0 + 256],
                                start=(cc == 0),
                                stop=(cc == 1),
                                tile_position=(0, 32 * i),
                            )
                    nc.vector.tensor_copy(
                        gT4_sb[:, qb, hq * 256:(hq + 1) * 256], st[:, 0:256]
                    )

            def proj_f(s):
                # One tile computes all four chunks of span s across the four
                # col groups; group i reads a 128i-shifted moving operand so
                # chunk 4s+i lands in cols 0:128 of rows 32i — the
                # de-interleave is two partition-aligned [64,128] copies.
                # (The very last group would read past the end of hT, so it
                # uses an N=128 matmul instead.)
                st = otile(f"fp{s}")
                for i in range(4):
                    m0 = s * 512 + 128 * i
                    nw = 256 if m0 + 256 <= N else 128
                    for cc in range(2):
                        nc.tensor.matmul(
                            st[32 * i:32 * (i + 1), 0:nw],
                            fwg_sb[:, cc * 2 * D:cc * 2 * D + D],
                            hT_sb[cc][:, m0:m0 + nw],
                            start=(cc == 0),
                            stop=(cc == 1),
                            tile_position=(0, 32 * i),
                        )
                nc.vector.tensor_copy(fT4_sb[0:64, s, :], st[0:64, 0:128])
                nc.vector.tensor_copy(fT4_sb[64:128, s, :], st[64:128, 0:128])

            proj_g(0)
            for s in range(8):
                proj_f(s)
            dma_gated_inputs()

            def pv(o_ps, e_ap, k):
                for i in range(NQSUB):
                    nc.tensor.matmul(
                        o_ps[i][:, :],
                        e_ap[:, i * 128:(i + 1) * 128],
                        hr_blk[k // 4][:, k % 4, :],
                        start=(k == 0),
                        stop=(k == NKC - 1),
                    )

            def norm_out(qb, o_ps):
                for i in range(NQSUB):
                    rec = r_pool.tile([128, 1], F32, tag="rec", name=f"rec{qb}_{i}")
                    nc.vector.reciprocal(rec[:, :], o_ps[i][:, C:C + 1])
                    out_sb = out_pool.tile([128, C], F32, tag="ob", name=f"ob{qb}_{i}")
                    nc.vector.tensor_scalar_mul(out_sb[:, :], o_ps[i][:, 0:C], rec[:, :])
                    r0 = qb * QBLK + i * 128
                    nc.sync.dma_start(out=o[r0:r0 + 128, :], in_=out_sb[:, :])

            # --- attention: steps of two chunk pairs, pipelined ACROSS query
            # blocks (the QK prefetch crosses qblock boundaries, so the PE
            # never drains between blocks).
            # step pipeline: [QK pair, QK pair](t+1) -> [exp, exp](t) -> [16x PV](t)
            with tc.tile_pool(name="sps", bufs=2, space="PSUM") as s_pool:
                def proj_g2(qb):
                    # gT for later query blocks, computed mid-attention in an
                    # s-pool bank (emission position keeps it late in the PE
                    # queue, so a late xT arrival cannot stall attention).
                    st = s_pool.tile([128, 2, QBLK], F32, tag="s", name=f"gp{qb}")
                    for i in range(4):
                        for cc in range(2):
                            nc.tensor.matmul(
                                st[32 * i:32 * (i + 1), 0, :],
                                fwg_sb[:, cc * 2 * D + D:cc * 2 * D + 2 * D],
                                xT_sb[cc][:, qb * QBLK:(qb + 1) * QBLK],
                                start=(cc == 0),
                                stop=(cc == 1),
                                tile_position=(0, 32 * i),
                            )
                    nc.vector.tensor_copy(gT4_sb[:, qb, :], st[:, 0, :])

                def qk_pair(p):
                    qb, g = divmod(p, NP)
                    s_ps = s_pool.tile([128, 2, QBLK], F32, tag="s", name=f"sps{qb}_{g}")
                    r0 = 64 * (g % 2)
                    for half in range(2):
                        rb = r0 + 32 * half
                        nc.tensor.matmul(
                            s_ps[:, half, :],
                            fT4_sb[rb:rb + 32, g // 2, :],
                            gT4_sb[rb:rb + 32, qb, :],
                            start=True,
                            stop=True,
                            tile_position=(rb, 0),
                        )
                    return s_ps

                NPAIRS = NQB * NP
                o_ps = None
                prev = [(0, qk_pair(0)), (1, qk_pair(1))]
                for t in range(NPAIRS // 2):
                    if 2 * t in (8, 24, 40):
                        proj_g2(2 * t // 16 + 1)
                    nxt = None
                    if 2 * t + 2 < NPAIRS:
                        nxt = [(2 * t + 2, qk_pair(2 * t + 2)), (2 * t + 3, qk_pair(2 * t + 3))]
                    es = []
                    for p, s_ps in prev:
                        qb, g = divmod(p, NP)
                        e_sb = e_pool.tile([128, 2, QBLK], BF16, tag="e", name=f"e{qb}_{g}")
                        nc.scalar.activation(e_sb[:, :, :], s_ps[:, :, :], Exp, bias=zbias[:, :])
                        es.append((p, e_sb))
                    for p, e in es:
                        qb, g = divmod(p, NP)
                        if g == 0:
                            o_ps = [
                                o_pool.tile([128, C + 2], F32, tag=f"o{i}", name=f"ops{qb}_{i}")
                                for i in range(NQSUB)
                            ]
                        for half in range(2):
                            pv(o_ps, e[:, half, :], 2 * g + half)
                        if g == NP - 1:
                            norm_out(qb, o_ps)
                    prev = nxt

    nc.finalize()
    return nc


_CACHE: dict = {}


def _get_nc() -> bass.Bass:
    if "nc" not in _CACHE:
        _CACHE["nc"] = _build()
    return _CACHE["nc"]


def _prep_batch(hf_b):
    """Per-batch host prep shared by both query-half cores."""
    hT = np.ascontiguousarray(hf_b.T.astype(np.float16))              # [C, N]
    aug = np.empty((N, C + 2), dtype=ml_dtypes.bfloat16)
    aug[:, 0:C] = hf_b.astype(ml_dtypes.bfloat16)
    aug[:, C] = 1.0
    aug[:, C + 1] = 0.0
    # chunk k = 4*blk + j holds keys 128k..128k+127: [blk, j, p, c] -> [blk, p, j, c]
    hR = np.ascontiguousarray(
        aug.reshape(NKC // 4, 4, 128, C + 2).transpose(0, 2, 1, 3).reshape(N, C + 2)
    )
    return hT, hR


def _shard(x, input_h, f_w, g_w):
    xf = np.asarray(x, dtype=np.float32).reshape(B, N, C)
    hf = np.asarray(input_h, dtype=np.float32).reshape(B, N, C)
    fwf = np.asarray(f_w, dtype=np.float32).reshape(C, D)
    gwf = np.asarray(g_w, dtype=np.float32).reshape(C, D)
    fwg = np.empty((128, 4 * D), dtype=np.float16)
    for cc in range(2):
        fwg[:, cc * 2 * D:cc * 2 * D + D] = fwf[cc * 128:(cc + 1) * 128, :]
        fwg[:, cc * 2 * D + D:cc * 2 * D + 2 * D] = gwf[cc * 128:(cc + 1) * 128, :]
    per_batch = [_prep_batch(hf[b]) for b in range(B)]
    in_maps = []
    for c in range(NCORES):
        b, half = divmod(c, SHARDS_PER_BATCH)
        hTb, hRb = per_batch[b]
        xTc = np.ascontiguousarray(
            xf[b, half * NQ:(half + 1) * NQ].T.astype(np.float16)
        )
        in_maps.append({"xT": xTc, "hT": hTb, "hR": hRb, "fwg": fwg})
    return in_maps


def _gather(results, x):
    of = np.empty((B, N, C), np.float32)
    for c in range(NCORES):
        b, half = divmod(c, SHARDS_PER_BATCH)
        of[b, half * NQ:(half + 1) * NQ] = results[c]["o"]
    o4 = of.reshape(B, W, W, C)
    x4 = np.asarray(x, dtype=np.float32).reshape(B, W, W, C)
    return np.concatenate([o4, x4], axis=-1)


def run(inputs: dict, trace: bool = False):
    """Run the kernel; returns (full_output, BassKernelResults)."""
    in_maps = _shard(**inputs)
    res = run_bass_kernel_spmd(_get_nc(), in_maps, list(range(NCORES)), trace=trace)
    out = _gather(res.results, inputs["x"])
    return out, res


def kernel(**inputs) -> np.ndarray:
    out, _ = run(inputs, trace=False)
    return out
